# revision 25
# baseline (speedup 1.0000x reference)
"""MultiHead HGNN attention (B=2, S=4096, D=256, H=4) on 8 TRN2 NeuronCores.

Sharding: query rows split 8 ways (512 rows/core); every core computes all
batches/heads for its query block. Scores are built k-major (scores^T) so
probs@V needs no transposes.

Key performance choices (all verified against hardware traces):
- G^T is pretransposed on host and sent as f32: the score*G multiply is a
  same-dtype f32 DVE op (~600ns per [128,512]); mixed f32*bf16 TensorTensor
  measured ~2x slower on hardware.
- The softmax exp runs per chunk as one [128,1024] ACT op (~1.04us); K/V
  PSUM->SBUF staging copies also run on ACT (Exp/Copy share one activation
  table, so no table reloads between them).
- Score matmuls are issued one kc-iteration ahead of the probs@V matmuls so
  the PE never head-of-line blocks on the exp; next-pair K/V projection
  groups are interleaved into the kc loop with their ACT copies trailing one
  iteration behind their matmuls.
- Batch 0's output projection is deferred into pair 2's kc loop (it
  otherwise stalls the PE queue ~20us waiting on the softmax denominators).
- DMA issue order is startup-critical-first: xqt+wq (Q projections), first
  kc chunks of x^T/G^T (pair-0 staging), then the rest streams under
  compute.
- Softmax denominators ride as an extra ones-column in the V operand; the
  reciprocal runs on a DMA-reshaped [128,8] layout (a 1-partition [1,512]
  reciprocal measured 3.3us on DVE). Bias is added on host.
"""

import contextlib
import ctypes
import sys
import types

import numpy as np

sys.path.insert(0, "/opt/trn_rl_repo")


def _install_axon_hooks():
    """The agent image's antenv lacks axon_hooks; provide it so bass_utils can
    NTFF-profile under axon. Harmless when profiling is never requested."""
    if "antenv.axon_hooks" in sys.modules:
        return
    try:
        import antenv
    except ImportError:
        return
    mod = types.ModuleType("antenv.axon_hooks")
    holder = {}
    mod.set_axon_ntff_profile_hook = lambda h: holder.__setitem__("h", h)
    mod.get_axon_ntff_profile_hook = lambda: holder.get("h")
    sys.modules["antenv.axon_hooks"] = mod
    antenv.axon_hooks = mod
    try:
        lib = ctypes.CDLL("/opt/axon/libaxon_pjrt.so")
    except OSError:
        return
    if not hasattr(lib, "axon_start_nrt_profile"):
        return
    lib.axon_start_nrt_profile.argtypes = [ctypes.POINTER(ctypes.c_int64), ctypes.c_size_t]
    lib.axon_start_nrt_profile.restype = ctypes.c_int64
    lib.axon_stop_nrt_profile.argtypes = [ctypes.c_char_p]
    lib.axon_stop_nrt_profile.restype = ctypes.c_int64

    @contextlib.contextmanager
    def _hook(output_dir, device_ids):
        import jax

        jax.devices()
        if device_ids:
            ids = (ctypes.c_int64 * len(device_ids))(*device_ids)
            rc = lib.axon_start_nrt_profile(ids, len(device_ids))
        else:
            rc = lib.axon_start_nrt_profile(None, 0)
        if rc != 0:
            raise RuntimeError(f"axon_start_nrt_profile rc={rc}")
        try:
            yield
        finally:
            n = lib.axon_stop_nrt_profile(str(output_dir).encode())
            print(f"profile: {n} file(s) written to {output_dir}")

    mod.set_axon_ntff_profile_hook(_hook)


_install_axon_hooks()

B, S, D, H, HD = 2, 4096, 256, 4, 64
NCORES = 8
QR = S // NCORES          # 512 query rows per core
KC = S // 128             # 32 key chunks of 128
SCALE = 1.0 / np.sqrt(HD)

_BUILT = {}


def build_bass():
    if "nc" in _BUILT:
        return _BUILT["nc"]

    import concourse.tile as tile
    from concourse import bacc, mybir

    f32, bf16 = mybir.dt.float32, mybir.dt.bfloat16
    af = mybir.ActivationFunctionType

    nc = bacc.Bacc("TRN2", target_bir_lowering=False, debug=False, num_devices=NCORES)

    xt_in = nc.dram_tensor("xt", [B, 2, 128, S], bf16, kind="ExternalInput")
    xqt_in = nc.dram_tensor("xqt", [B, 2, 128, QR], bf16, kind="ExternalInput")
    # gt[p, kc, q] = G[q0 + q, kc*128 + p]; f32 so the score multiply is a
    # same-dtype DVE op (mixed f32xbf16 TensorTensor runs ~2x slower)
    gt_in = nc.dram_tensor("gt", [128, KC, QR], f32, kind="ExternalInput")
    wq_in = nc.dram_tensor("wq", [2, 128, 256], bf16, kind="ExternalInput")
    wk_in = nc.dram_tensor("wk", [2, 128, 256], bf16, kind="ExternalInput")
    wv_in = nc.dram_tensor("wv", [2, 128, 260], bf16, kind="ExternalInput")
    wo_in = nc.dram_tensor("wo", [H, 64, 256], bf16, kind="ExternalInput")
    out_dram = nc.dram_tensor("out", [B, QR, 256], f32, kind="ExternalOutput")

    with tile.TileContext(nc) as tc, contextlib.ExitStack() as ctx:
        cp = ctx.enter_context(tc.tile_pool(name="const", bufs=1))
        # 3 rotating slots x 2 banks: score chunks + K/V/Q/O staging
        ps_big = ctx.enter_context(tc.tile_pool(name="ps_big", bufs=3, space="PSUM"))
        ps_ct = ctx.enter_context(tc.tile_pool(name="ps_ct", bufs=1, space="PSUM"))

        # ---- DMA issue order = startup priority: xqt+wq (Q proj), then
        # ---- wk/wv + xt b0 + first G piece (pair-0 staging + loop start),
        # ---- then everything else streaming under the compute
        wq_sb = cp.tile([128, 2, 256], bf16, tag="wq")
        wk_sb = cp.tile([128, 2, 256], bf16, tag="wk")
        wv_sb = cp.tile([128, 2, 260], bf16, tag="wv")
        xt_sb = [[cp.tile([128, S], bf16, tag=f"xt{b}{ic}", name=f"xt{b}{ic}") for ic in range(2)] for b in range(B)]
        gt_sb = cp.tile([128, KC, QR], f32, tag="gt")
        qts = [[cp.tile([128, QR], bf16, tag=f"qt{b}{hp}", name=f"qt{b}{hp}") for hp in range(2)] for b in range(B)]
        wo_sb = [cp.tile([64, 256], bf16, tag=f"wo{h}", name=f"wo{h}") for h in range(H)]

        with tc.tile_pool(name="xqp", bufs=1) as xqp:
            xqt_sb = xqp.tile([128, B, 2, QR], bf16, tag="xqt")
            for b in range(B):
                for ic in range(2):
                    nc.sync.dma_start(xqt_sb[:, b, ic, :], xqt_in[b, ic])
            for ic in range(2):
                nc.sync.dma_start(wq_sb[:, ic, :], wq_in[ic])
                nc.sync.dma_start(wk_sb[:, ic, :], wk_in[ic])
                nc.sync.dma_start(wv_sb[:, ic, :], wv_in[ic])
            # first kc chunks of x^T/G^T land first so pair-0 staging and the
            # first score chunks start as early as possible
            for ic in range(2):
                nc.sync.dma_start(xt_sb[0][ic][:, 0:1024], xt_in[0, ic, :, 0:1024])
            nc.sync.dma_start(gt_sb[:, 0:4, :], gt_in[:, 0:4, :])
            for ic in range(2):
                nc.sync.dma_start(xt_sb[0][ic][:, 1024:S], xt_in[0, ic, :, 1024:S])
            # Q projections for all pairs
            for b in range(B):
                for hp in range(2):
                    aux = ps_big.tile([128, 2, 512], f32, tag="sc", name="auxq")
                    for ic in range(2):
                        nc.tensor.matmul(
                            aux[:, 0, :QR], wq_sb[:, ic, hp * 128:(hp + 1) * 128],
                            xqt_sb[:, b, ic, :], start=(ic == 0), stop=(ic == 1),
                        )
                    nc.vector.tensor_copy(qts[b][hp][:], aux[:, 0, :QR])
            # remaining loads stream in while compute ramps
            for kp in range(1, 8):
                nc.sync.dma_start(gt_sb[:, kp * 4:(kp + 1) * 4, :], gt_in[:, kp * 4:(kp + 1) * 4, :])
            for ic in range(2):
                nc.sync.dma_start(xt_sb[1][ic][:], xt_in[1, ic])
            for h in range(H):
                nc.sync.dma_start(wo_sb[h][:], wo_in[h])

        ktp = ctx.enter_context(tc.tile_pool(name="ktp", bufs=2))
        vap = ctx.enter_context(tc.tile_pool(name="vap", bufs=2))
        ttp = ctx.enter_context(tc.tile_pool(name="ttp", bufs=4))
        pp = ctx.enter_context(tc.tile_pool(name="pp", bufs=6))
        otp = ctx.enter_context(tc.tile_pool(name="otp", bufs=2))
        rp = ctx.enter_context(tc.tile_pool(name="rp", bufs=2))
        csp = ctx.enter_context(tc.tile_pool(name="csp", bufs=4))

        PAIRS = [(b, hp) for b in range(B) for hp in range(2)]

        def stage_kv(b, hp, kt, va):
            """Return a list of (matmul_thunk, copy_thunk) staging groups.
            The copy is issued 1 kc-iteration after the matmuls so the ACT
            queue never head-of-line blocks on a not-yet-run PE matmul."""
            groups = []

            def k_group(sc4):
                aux_box = {}

                def mms():
                    aux = ps_big.tile([128, 2, 512], f32, tag="sc", name="auxk")
                    for half in range(2):
                        for ic in range(2):
                            nc.tensor.matmul(
                                aux[:, half, :], wk_sb[:, ic, hp * 128:(hp + 1) * 128],
                                xt_sb[b][ic][:, (sc4 * 2 + half) * 512:(sc4 * 2 + half + 1) * 512],
                                start=(ic == 0), stop=(ic == 1),
                            )
                    aux_box["t"] = aux

                def cp_():
                    nc.scalar.copy(kt[:, sc4 * 1024:(sc4 + 1) * 1024], aux_box["t"][:, :, :])
                return (mms, cp_)

            def v_group(g):
                aux_box = {}

                def mms():
                    aux = ps_big.tile([128, 2, 512], f32, tag="sc", name="auxv")
                    for q4 in range(4):
                        kcj = g * 4 + q4
                        for ic in range(2):
                            nc.tensor.matmul(
                                aux[:, q4 // 2, (q4 % 2) * 130:(q4 % 2) * 130 + 130],
                                xt_sb[b][ic][:, kcj * 128:(kcj + 1) * 128],
                                wv_sb[:, ic, hp * 130:(hp + 1) * 130],
                                start=(ic == 0), stop=(ic == 1),
                            )
                    aux_box["t"] = aux

                def cp_():
                    nc.scalar.copy(
                        va[:, g * 4:(g + 1) * 4, :].rearrange("p (a c) d -> p a (c d)", a=2),
                        aux_box["t"][:, :, 0:260],
                    )
                    nc.vector.memset(va[:, g * 4:(g + 1) * 4, 64:65], 1.0)
                    nc.vector.memset(va[:, g * 4:(g + 1) * 4, 129:130], 1.0)
                return (mms, cp_)

            # K/V interleaved so each chunk's operands are staged a couple of
            # iterations before the consuming kc needs them
            order = [("k", 0), ("v", 0), ("k", 1), ("v", 1), ("v", 2), ("k", 2),
                     ("v", 3), ("k", 3), ("v", 4), ("v", 5), ("v", 6), ("v", 7)]
            for kind, idx in order:
                groups.append(k_group(idx) if kind == "k" else v_group(idx))
            return groups

        def outproj_group(b, qs):
            """One out-projection q-chunk as a deferrable (mms, copy) group."""
            aux_box = {}

            def mms():
                op = ps_big.tile([128, 2, 512], f32, tag="sc", name="auxo")
                for h in range(H):
                    nc.tensor.matmul(
                        op[:, 0, 0:256], ctf[b][h][:, qs * 128:(qs + 1) * 128],
                        wo_sb[h][:], start=(h == 0), stop=(h == H - 1),
                    )
                aux_box["t"] = op

            def cp_():
                ot = otp.tile([128, 256], f32, tag="ot")
                nc.scalar.copy(ot[:], aux_box["t"][:, 0, 0:256])
                nc.sync.dma_start(out_dram[b, qs * 128:(qs + 1) * 128, :], ot[:])
            return (mms, cp_)

        # K/V tiles per pair index (rotating pools, 2 bufs each)
        kts = {}
        vas = {}

        # pair 0: stage only the first K and V groups up front; the rest
        # interleave into pair 0's own kc loop
        kts[0] = ktp.tile([128, S], bf16, tag="kt", name="kt0")
        vas[0] = vap.tile([128, KC, 130], bf16, tag="va", name="va0")
        groups0 = stage_kv(PAIRS[0][0], PAIRS[0][1], kts[0], vas[0])
        for mms, cp_ in groups0[:2]:
            mms()
            cp_()
        carry = groups0[2:]

        ctf = [[None] * H for _ in range(B)]

        for pi, (b, hp) in enumerate(PAIRS):
            qt = qts[b][hp]
            kt, va = kts[pi], vas[pi]

            # pending work interleaved into this pair's kc loop: leftover own
            # staging (pair 0), the next pair's staging, and the previous
            # batch's deferred out-projection
            pending = list(carry)
            carry = []
            if pi + 1 < len(PAIRS):
                nb, nhp = PAIRS[pi + 1]
                kts[pi + 1] = ktp.tile([128, S], bf16, tag="kt", name=f"kt{pi + 1}")
                vas[pi + 1] = vap.tile([128, KC, 130], bf16, tag="va", name=f"va{pi + 1}")
                pending += stage_kv(nb, nhp, kts[pi + 1], vas[pi + 1])
            if pi == 2:
                pending += [outproj_group(0, qs) for qs in range(QR // 128)]

            # ---- main loop over key chunks; score matmuls are issued one
            # ---- iteration ahead so the PE never waits on exp before them
            ct0 = ps_ct.tile([65, QR], f32, tag="ct0")
            ct1 = ps_ct.tile([65, QR], f32, tag="ct1")
            scps = {}

            def issue_sc(kc):
                scp = ps_big.tile([128, 2, QR], f32, tag="sc", name="scp")
                nc.tensor.matmul(
                    scp[:, 0, :], kt[0:64, kc * 128:(kc + 1) * 128], qt[0:64, :],
                    start=True, stop=True, tile_position=(0, 0),
                )
                nc.tensor.matmul(
                    scp[:, 1, :], kt[64:128, kc * 128:(kc + 1) * 128], qt[64:128, :],
                    start=True, stop=True, tile_position=(64, 0),
                )
                scps[kc] = scp

            issue_sc(0)
            due_copy = None
            slot0 = 0 if pi == 0 else 2
            stride = 2 if len(pending) <= 16 else 1
            for kc in range(KC):
                if kc + 1 < KC:
                    issue_sc(kc + 1)
                scp = scps.pop(kc)
                tt = ttp.tile([128, 2, QR], f32, tag="tt")
                for j in range(2):
                    nc.vector.tensor_mul(tt[:, j, :], scp[:, j, :], gt_sb[:, kc, :])
                pt = pp.tile([128, 2, QR], bf16, tag="pt")
                nc.scalar.activation(pt[:, :, :], tt[:, :, :], af.Exp)
                nc.tensor.matmul(
                    ct0[:, :], va[:, kc, 0:65], pt[:, 0, :],
                    start=(kc == 0), stop=(kc == KC - 1),
                )
                nc.tensor.matmul(
                    ct1[:, :], va[:, kc, 65:130], pt[:, 1, :],
                    start=(kc == 0), stop=(kc == KC - 1),
                )
                # interleave pending groups, one per iteration; each group's
                # copy trails its matmuls by one iteration
                if due_copy is not None:
                    due_copy()
                    due_copy = None
                if kc >= slot0 and (kc - slot0) % stride == 0:
                    gi = (kc - slot0) // stride
                    if gi < len(pending):
                        mms, cp_ = pending[gi]
                        mms()
                        due_copy = cp_

            if due_copy is not None:
                due_copy()
            for gi in range((KC - slot0 + stride - 1) // stride, len(pending)):
                mms, cp_ = pending[gi]
                mms()
                cp_()

            # ---- evacuate ctx^T to SBUF fast (frees ct banks), then the
            # ---- per-pair 1/denom dance: DMA-reshape [1,1024]->[128,8] so the
            # ---- reciprocal runs across partitions (a 1-partition reciprocal
            # ---- measured 3.3us; this way it is ~70ns + DMA latency)
            c0 = csp.tile([64, QR], f32, tag="cs", name=f"cs{pi}_0")
            c1 = csp.tile([64, QR], f32, tag="cs", name=f"cs{pi}_1")
            den = rp.tile([1, 2, QR], f32, tag="den")
            nc.scalar.copy(c0[:], ct0[0:64, :])
            nc.scalar.copy(c1[:], ct1[0:64, :])
            nc.vector.tensor_copy(den[0:1, 0, :], ct0[64:65, :])
            nc.vector.tensor_copy(den[0:1, 1, :], ct1[64:65, :])
            denp = rp.tile([128, 8], f32, tag="denp")
            nc.sync.dma_start(denp[:, :], den[0:1, :, :])
            recp = rp.tile([128, 8], f32, tag="recp")
            nc.vector.reciprocal(recp[:], denp[:])
            rec = rp.tile([1, 2, QR], f32, tag="rec")
            nc.sync.dma_start(rec[0:1, :, :], recp[:, :])
            for j, cj in ((0, c0), (1, c1)):
                bcb = rp.tile([64, QR], f32, tag="bcb")
                nc.sync.dma_start(bcb[:, :], rec[0:1, j, :].rearrange("p (o q) -> p o q", o=1).broadcast_to([1, 64, QR]))
                cf = cp.tile([64, QR], bf16, tag=f"ctf{b}_{2 * hp + j}", name=f"ctf{b}_{2 * hp + j}")
                nc.vector.tensor_mul(cf[:], cj[:], bcb[:, :])
                ctf[b][2 * hp + j] = cf

            # ---- batch 1's out-projection runs at the tail (batch 0's was
            # ---- deferred into pair 2's loop)
            if pi == 3:
                for qs in range(QR // 128):
                    mms, cp_ = outproj_group(1, qs)
                    mms()
                    cp_()

    nc.compile()
    _BUILT["nc"] = nc
    return nc


def host_inputs(x, G, Wq, Wk, Wv, Wo, bo, b_extra):
    """Build the per-core input maps (layout prep + query-row sharding)."""
    import ml_dtypes

    f = np.float32
    bf = ml_dtypes.bfloat16
    x = np.asarray(x, f)
    G = np.asarray(G, f)
    xt = np.ascontiguousarray(x.transpose(0, 2, 1)).reshape(B, 2, 128, S).astype(bf)
    wq = np.ascontiguousarray(np.asarray(Wq, f).T * SCALE).reshape(2, 128, 256).astype(bf)
    wk = np.ascontiguousarray(np.asarray(Wk, f).T).reshape(2, 128, 256).astype(bf)
    wvt = np.asarray(Wv, f).T  # [256 in, 256 out]
    wv = np.zeros((2, 128, 260), f)
    for hp in range(2):
        wv[:, :, hp * 130:hp * 130 + 64] = wvt[:, hp * 128:hp * 128 + 64].reshape(2, 128, 64)
        wv[:, :, hp * 130 + 65:hp * 130 + 129] = wvt[:, hp * 128 + 64:hp * 128 + 128].reshape(2, 128, 64)
    wv = wv.astype(bf)
    wo = np.ascontiguousarray(np.asarray(Wo, f).T).reshape(H, 64, 256).astype(bf)

    shared = {"xt": xt, "wq": wq, "wk": wk, "wv": wv, "wo": wo}
    in_maps = []
    for c in range(NCORES):
        q0 = c * QR
        m = dict(shared)
        # gt[p, kc, q] = G[q0+q, kc*128+p]  (f32)
        gslice = np.ascontiguousarray(G[q0:q0 + QR, :].T)            # [S, QR]
        m["gt"] = np.ascontiguousarray(gslice.reshape(KC, 128, QR).transpose(1, 0, 2))
        m["xqt"] = np.ascontiguousarray(xt[:, :, :, q0:q0 + QR])
        in_maps.append(m)
    return in_maps


def run(in_maps, trace=False):
    from concourse.bass_utils import run_bass_kernel_spmd

    nc = build_bass()
    return run_bass_kernel_spmd(nc, in_maps, core_ids=list(range(NCORES)), trace=trace)


def kernel(x, G, Wq, Wk, Wv, Wo, bo, b_extra):
    in_maps = host_inputs(x, G, Wq, Wk, Wv, Wo, bo, b_extra)
    res = run(in_maps, trace=False)
    out = np.concatenate([res.results[c]["out"] for c in range(NCORES)], axis=1)
    bias = (np.asarray(bo, np.float32) + np.asarray(b_extra, np.float32)).reshape(1, 1, 256)
    return out.astype(np.float32) + bias


# revision 31
# speedup vs baseline: 1.1066x; 1.1066x over previous
"""MultiHead HGNN attention (B=2, S=4096, D=256, H=4) on 8 TRN2 NeuronCores.

Sharding: query rows split 8 ways (512 rows/core); every core computes all
batches/heads for its query block. Scores are built k-major (scores^T) so
probs@V needs no transposes.

Key performance choices (all verified against hardware traces):
- G^T is pretransposed on host and sent as f32: the score*G multiply is a
  same-dtype f32 DVE op (~600ns per [128,512]); mixed f32*bf16 TensorTensor
  measured ~2x slower on hardware.
- The softmax exp runs per chunk as one [128,1024] ACT op (~1.04us); K/V
  PSUM->SBUF staging copies also run on ACT (Exp/Copy share one activation
  table, so no table reloads between them).
- Score matmuls are issued one kc-iteration ahead of the probs@V matmuls so
  the PE never head-of-line blocks on the exp; next-pair K/V projection
  groups are interleaved into the kc loop with their ACT copies trailing one
  iteration behind their matmuls.
- Batch 0's output projection is deferred into pair 2's kc loop (it
  otherwise stalls the PE queue ~20us waiting on the softmax denominators).
- DMA issue order is startup-critical-first: xqt+wq (Q projections), first
  kc chunks of x^T/G^T (pair-0 staging), then the rest streams under
  compute.
- Softmax denominators ride as an extra ones-column in the V operand; the
  reciprocal runs on a DMA-reshaped [128,8] layout (a 1-partition [1,512]
  reciprocal measured 3.3us on DVE). Bias is added on host.
"""

import contextlib
import ctypes
import sys
import types

import numpy as np

sys.path.insert(0, "/opt/trn_rl_repo")


def _install_axon_hooks():
    """The agent image's antenv lacks axon_hooks; provide it so bass_utils can
    NTFF-profile under axon. Harmless when profiling is never requested."""
    if "antenv.axon_hooks" in sys.modules:
        return
    try:
        import antenv
    except ImportError:
        return
    mod = types.ModuleType("antenv.axon_hooks")
    holder = {}
    mod.set_axon_ntff_profile_hook = lambda h: holder.__setitem__("h", h)
    mod.get_axon_ntff_profile_hook = lambda: holder.get("h")
    sys.modules["antenv.axon_hooks"] = mod
    antenv.axon_hooks = mod
    try:
        lib = ctypes.CDLL("/opt/axon/libaxon_pjrt.so")
    except OSError:
        return
    if not hasattr(lib, "axon_start_nrt_profile"):
        return
    lib.axon_start_nrt_profile.argtypes = [ctypes.POINTER(ctypes.c_int64), ctypes.c_size_t]
    lib.axon_start_nrt_profile.restype = ctypes.c_int64
    lib.axon_stop_nrt_profile.argtypes = [ctypes.c_char_p]
    lib.axon_stop_nrt_profile.restype = ctypes.c_int64

    @contextlib.contextmanager
    def _hook(output_dir, device_ids):
        import jax

        jax.devices()
        if device_ids:
            ids = (ctypes.c_int64 * len(device_ids))(*device_ids)
            rc = lib.axon_start_nrt_profile(ids, len(device_ids))
        else:
            rc = lib.axon_start_nrt_profile(None, 0)
        if rc != 0:
            raise RuntimeError(f"axon_start_nrt_profile rc={rc}")
        try:
            yield
        finally:
            n = lib.axon_stop_nrt_profile(str(output_dir).encode())
            print(f"profile: {n} file(s) written to {output_dir}")

    mod.set_axon_ntff_profile_hook(_hook)


_install_axon_hooks()

B, S, D, H, HD = 2, 4096, 256, 4, 64
NCORES = 8
QR = S // NCORES          # 512 query rows per core
KC = S // 128             # 32 key chunks of 128
SCALE = 1.0 / np.sqrt(HD)

_BUILT = {}


def build_bass():
    if "nc" in _BUILT:
        return _BUILT["nc"]

    import concourse.tile as tile
    from concourse import bacc, mybir

    f32, bf16 = mybir.dt.float32, mybir.dt.bfloat16
    af = mybir.ActivationFunctionType

    nc = bacc.Bacc("TRN2", target_bir_lowering=False, debug=False, num_devices=NCORES)

    xt_in = nc.dram_tensor("xt", [B, 2, 128, S], bf16, kind="ExternalInput")
    xqt_in = nc.dram_tensor("xqt", [B, 2, 128, QR], bf16, kind="ExternalInput")
    # gt[p, kc, q] = G[q0 + q, kc*128 + p]; f32 so the score multiply is a
    # same-dtype DVE op (mixed f32xbf16 TensorTensor runs ~2x slower)
    gt_in = nc.dram_tensor("gt", [128, KC, QR], f32, kind="ExternalInput")
    wq_in = nc.dram_tensor("wq", [2, 128, 256], bf16, kind="ExternalInput")
    wk_in = nc.dram_tensor("wk", [2, 128, 256], bf16, kind="ExternalInput")
    wv_in = nc.dram_tensor("wv", [2, 128, 260], bf16, kind="ExternalInput")
    wo_in = nc.dram_tensor("wo", [H, 64, 256], bf16, kind="ExternalInput")
    out_dram = nc.dram_tensor("out", [B, QR, 256], f32, kind="ExternalOutput")

    with tile.TileContext(nc) as tc, contextlib.ExitStack() as ctx:
        cp = ctx.enter_context(tc.tile_pool(name="const", bufs=1))
        # 3 rotating slots x 2 banks: score chunks + K/V/Q/O staging
        ps_big = ctx.enter_context(tc.tile_pool(name="ps_big", bufs=3, space="PSUM"))
        ps_ct = ctx.enter_context(tc.tile_pool(name="ps_ct", bufs=1, space="PSUM"))

        # ---- DMA issue order = startup priority: xqt+wq (Q proj), then
        # ---- wk/wv + xt b0 + first G piece (pair-0 staging + loop start),
        # ---- then everything else streaming under the compute
        wq_sb = cp.tile([128, 2, 256], bf16, tag="wq")
        wk_sb = cp.tile([128, 2, 256], bf16, tag="wk")
        wv_sb = cp.tile([128, 2, 260], bf16, tag="wv")
        xt_sb = [[cp.tile([128, S], bf16, tag=f"xt{b}{ic}", name=f"xt{b}{ic}") for ic in range(2)] for b in range(B)]
        gt_sb = cp.tile([128, KC, QR], f32, tag="gt")
        qts = [[cp.tile([128, QR], bf16, tag=f"qt{b}{hp}", name=f"qt{b}{hp}") for hp in range(2)] for b in range(B)]
        wo_sb = [cp.tile([64, 256], bf16, tag=f"wo{h}", name=f"wo{h}") for h in range(H)]

        ones1 = cp.tile([1, 64], f32, tag="ones1")
        nc.vector.memset(ones1[:], 1.0)

        with tc.tile_pool(name="xqp", bufs=1) as xqp:
            xqt_sb = xqp.tile([128, B, 2, QR], bf16, tag="xqt")
            for b in range(B):
                for ic in range(2):
                    nc.sync.dma_start(xqt_sb[:, b, ic, :], xqt_in[b, ic])
            for ic in range(2):
                nc.sync.dma_start(wq_sb[:, ic, :], wq_in[ic])
                nc.sync.dma_start(wk_sb[:, ic, :], wk_in[ic])
                nc.sync.dma_start(wv_sb[:, ic, :], wv_in[ic])
            # first kc chunks of x^T/G^T land first so pair-0 staging and the
            # first score chunks start as early as possible
            for ic in range(2):
                nc.sync.dma_start(xt_sb[0][ic][:, 0:1024], xt_in[0, ic, :, 0:1024])
            nc.sync.dma_start(gt_sb[:, 0:4, :], gt_in[:, 0:4, :])
            for ic in range(2):
                nc.sync.dma_start(xt_sb[0][ic][:, 1024:S], xt_in[0, ic, :, 1024:S])
            # Q projections for all pairs
            for b in range(B):
                for hp in range(2):
                    aux = ps_big.tile([128, 2, 512], f32, tag="sc", name="auxq")
                    for ic in range(2):
                        nc.tensor.matmul(
                            aux[:, 0, :QR], wq_sb[:, ic, hp * 128:(hp + 1) * 128],
                            xqt_sb[:, b, ic, :], start=(ic == 0), stop=(ic == 1),
                        )
                    nc.vector.tensor_copy(qts[b][hp][:], aux[:, 0, :QR])
            # remaining loads stream in while compute ramps
            for kp in range(1, 8):
                nc.sync.dma_start(gt_sb[:, kp * 4:(kp + 1) * 4, :], gt_in[:, kp * 4:(kp + 1) * 4, :])
            for ic in range(2):
                nc.sync.dma_start(xt_sb[1][ic][:], xt_in[1, ic])
            for h in range(H):
                nc.sync.dma_start(wo_sb[h][:], wo_in[h])

        ktp = ctx.enter_context(tc.tile_pool(name="ktp", bufs=2))
        vap = ctx.enter_context(tc.tile_pool(name="vap", bufs=2))
        ttp = ctx.enter_context(tc.tile_pool(name="ttp", bufs=4))
        pp = ctx.enter_context(tc.tile_pool(name="pp", bufs=6))
        otp = ctx.enter_context(tc.tile_pool(name="otp", bufs=2))
        rp = ctx.enter_context(tc.tile_pool(name="rp", bufs=2))
        csp = ctx.enter_context(tc.tile_pool(name="csp", bufs=4))

        PAIRS = [(b, hp) for b in range(B) for hp in range(2)]

        def stage_kv(b, hp, kt, va):
            """Return a list of (matmul_thunk, copy_thunk) staging groups.
            The copy is issued 1 kc-iteration after the matmuls so the ACT
            queue never head-of-line blocks on a not-yet-run PE matmul."""
            groups = []

            def k_group(sc4):
                aux_box = {}

                def mms():
                    aux = ps_big.tile([128, 2, 512], f32, tag="sc", name="auxk")
                    for half in range(2):
                        for ic in range(2):
                            nc.tensor.matmul(
                                aux[:, half, :], wk_sb[:, ic, hp * 128:(hp + 1) * 128],
                                xt_sb[b][ic][:, (sc4 * 2 + half) * 512:(sc4 * 2 + half + 1) * 512],
                                start=(ic == 0), stop=(ic == 1),
                            )
                    aux_box["t"] = aux

                def cp_():
                    nc.scalar.copy(kt[:, sc4 * 1024:(sc4 + 1) * 1024], aux_box["t"][:, :, :])
                return (mms, cp_)

            def v_group(g):
                aux_box = {}

                def mms():
                    aux = ps_big.tile([128, 2, 512], f32, tag="sc", name="auxv")
                    for q4 in range(4):
                        kcj = g * 4 + q4
                        for ic in range(2):
                            nc.tensor.matmul(
                                aux[:, q4 // 2, (q4 % 2) * 130:(q4 % 2) * 130 + 130],
                                xt_sb[b][ic][:, kcj * 128:(kcj + 1) * 128],
                                wv_sb[:, ic, hp * 130:(hp + 1) * 130],
                                start=(ic == 0), stop=(ic == 1),
                            )
                    aux_box["t"] = aux

                def cp_():
                    nc.scalar.copy(
                        va[:, g * 4:(g + 1) * 4, :].rearrange("p (a c) d -> p a (c d)", a=2),
                        aux_box["t"][:, :, 0:260],
                    )
                    nc.vector.memset(va[:, g * 4:(g + 1) * 4, 64:65], 1.0)
                    nc.vector.memset(va[:, g * 4:(g + 1) * 4, 129:130], 1.0)
                return (mms, cp_)

            # K/V interleaved so each chunk's operands are staged a couple of
            # iterations before the consuming kc needs them
            order = [("k", 0), ("v", 0), ("k", 1), ("v", 1), ("v", 2), ("k", 2),
                     ("v", 3), ("k", 3), ("v", 4), ("v", 5), ("v", 6), ("v", 7)]
            for kind, idx in order:
                groups.append(k_group(idx) if kind == "k" else v_group(idx))
            return groups

        def outproj_group(b, qs):
            """One out-projection q-chunk as a deferrable (mms, copy) group."""
            aux_box = {}

            def mms():
                op = ps_big.tile([128, 2, 512], f32, tag="sc", name="auxo")
                for h in range(H):
                    nc.tensor.matmul(
                        op[:, 0, 0:256], ctf[b][h][:, qs * 128:(qs + 1) * 128],
                        wo_sb[h][:], start=(h == 0), stop=(h == H - 1),
                    )
                aux_box["t"] = op

            def cp_():
                ot = otp.tile([128, 256], f32, tag="ot")
                nc.scalar.copy(ot[:], aux_box["t"][:, 0, 0:256])
                nc.sync.dma_start(out_dram[b, qs * 128:(qs + 1) * 128, :], ot[:])
            return (mms, cp_)

        # K/V tiles per pair index (rotating pools, 2 bufs each)
        kts = {}
        vas = {}

        # pair 0: stage only the first K and V groups up front; the rest
        # interleave into pair 0's own kc loop
        kts[0] = ktp.tile([128, S], bf16, tag="kt", name="kt0")
        vas[0] = vap.tile([128, KC, 130], bf16, tag="va", name="va0")
        groups0 = stage_kv(PAIRS[0][0], PAIRS[0][1], kts[0], vas[0])
        for mms, cp_ in groups0[:2]:
            mms()
            cp_()
        carry = groups0[2:]

        ctf = [[None] * H for _ in range(B)]

        for pi, (b, hp) in enumerate(PAIRS):
            qt = qts[b][hp]
            kt, va = kts[pi], vas[pi]

            # pending work interleaved into this pair's kc loop: the next
            # pair's staging, the previous pair's deferred softmax-normalize
            # (a few slots in, so its rec DMA has landed), and batch 0's
            # deferred out-projection. For pair 0 it also holds pair 0's own
            # remaining staging.
            nxt = []
            if pi + 1 < len(PAIRS):
                nb, nhp = PAIRS[pi + 1]
                kts[pi + 1] = ktp.tile([128, S], bf16, tag="kt", name=f"kt{pi + 1}")
                vas[pi + 1] = vap.tile([128, KC, 130], bf16, tag="va", name=f"va{pi + 1}")
                nxt = stage_kv(nb, nhp, kts[pi + 1], vas[pi + 1])
            if pi == 0:
                pending = list(carry) + nxt
            else:
                pending = nxt[:4] + list(carry) + nxt[4:]
                if pi == 2:
                    pending += [outproj_group(0, qs) for qs in range(QR // 128)]
            carry = []

            # ---- main loop over key chunks; score matmuls are issued one
            # ---- iteration ahead so the PE never waits on exp before them
            ct0 = ps_ct.tile([65, QR], f32, tag="ct0")
            ct1 = ps_ct.tile([65, QR], f32, tag="ct1")
            scps = {}

            def issue_sc(kc):
                scp = ps_big.tile([128, 2, QR], f32, tag="sc", name="scp")
                nc.tensor.matmul(
                    scp[:, 0, :], kt[0:64, kc * 128:(kc + 1) * 128], qt[0:64, :],
                    start=True, stop=True, tile_position=(0, 0),
                )
                nc.tensor.matmul(
                    scp[:, 1, :], kt[64:128, kc * 128:(kc + 1) * 128], qt[64:128, :],
                    start=True, stop=True, tile_position=(64, 0),
                )
                scps[kc] = scp

            issue_sc(0)
            due_copy = None
            slot0 = 0 if pi == 0 else (8 if len(pending) <= 2 else 2)
            stride = 2 if len(pending) <= 16 else 1
            for kc in range(KC):
                if kc + 1 < KC:
                    issue_sc(kc + 1)
                scp = scps.pop(kc)
                tt = ttp.tile([128, 2, QR], f32, tag="tt")
                for j in range(2):
                    nc.vector.tensor_mul(tt[:, j, :], scp[:, j, :], gt_sb[:, kc, :])
                pt = pp.tile([128, 2, QR], bf16, tag="pt")
                nc.scalar.activation(pt[:, :, :], tt[:, :, :], af.Exp)
                nc.tensor.matmul(
                    ct0[:, :], va[:, kc, 0:65], pt[:, 0, :],
                    start=(kc == 0), stop=(kc == KC - 1),
                )
                nc.tensor.matmul(
                    ct1[:, :], va[:, kc, 65:130], pt[:, 1, :],
                    start=(kc == 0), stop=(kc == KC - 1),
                )
                # interleave pending groups, one per iteration; each group's
                # copy trails its matmuls by one iteration
                if due_copy is not None:
                    due_copy()
                    due_copy = None
                if kc >= slot0 and (kc - slot0) % stride == 0:
                    gi = (kc - slot0) // stride
                    if gi < len(pending):
                        mms, cp_ = pending[gi]
                        mms()
                        due_copy = cp_

            if due_copy is not None:
                due_copy()
            for gi in range((KC - slot0 + stride - 1) // stride, len(pending)):
                mms, cp_ = pending[gi]
                mms()
                cp_()

            # ---- evacuate ctx^T to SBUF fast (frees ct banks), then the
            # ---- per-pair 1/denom dance: DMA-reshape [1,1024]->[128,8] so the
            # ---- reciprocal runs across partitions (a 1-partition reciprocal
            # ---- measured 3.3us; this way it is ~70ns + one DMA round trip)
            c0 = csp.tile([64, QR], f32, tag="cs", name=f"cs{pi}_0")
            c1 = csp.tile([64, QR], f32, tag="cs", name=f"cs{pi}_1")
            den = rp.tile([1, 2, QR], f32, tag="den")
            nc.scalar.copy(c0[:], ct0[0:64, :])
            nc.scalar.copy(c1[:], ct1[0:64, :])
            nc.vector.tensor_copy(den[0:1, 0, :], ct0[64:65, :])
            nc.vector.tensor_copy(den[0:1, 1, :], ct1[64:65, :])
            denp = rp.tile([128, 8], f32, tag="denp")
            nc.sync.dma_start(denp[:, :], den[0:1, :, :])
            recp = rp.tile([128, 8], f32, tag="recp")
            nc.vector.reciprocal(recp[:], denp[:])
            rec = rp.tile([1, 2, QR], f32, tag="rec")
            nc.sync.dma_start(rec[0:1, :, :], recp[:, :])

            # the broadcast of 1/denom runs as a PE outer product (ones x rec)
            # into PSUM -- no broadcast DMA -- and the normalize multiplies are
            # deferred into the next pair's loop so the DVE queue never stalls
            def dance_fin(j, cj, b_, hp_):
                box = {}

                def mms():
                    bcb = ps_big.tile([128, 2, 512], f32, tag="sc", name="bcbp")
                    nc.tensor.matmul(bcb[0:64, 0, :], ones1[0:1, :], rec[0:1, j, :], start=True, stop=True)
                    box["t"] = bcb

                def cp_():
                    cf = cp.tile([64, QR], bf16, tag=f"ctf{b_}_{2 * hp_ + j}", name=f"ctf{b_}_{2 * hp_ + j}")
                    nc.vector.tensor_mul(cf[:], cj[:], box["t"][0:64, 0, :])
                    ctf[b_][2 * hp_ + j] = cf
                return (mms, cp_)

            fin = [dance_fin(0, c0, b, hp), dance_fin(1, c1, b, hp)]
            if pi < 3:
                carry = fin
            else:
                for mms, cp_ in fin:
                    mms()
                    cp_()
                # batch 1's out-projection runs at the tail (batch 0's was
                # deferred into pair 2's loop)
                for qs in range(QR // 128):
                    mms, cp_ = outproj_group(1, qs)
                    mms()
                    cp_()

    nc.compile()
    _BUILT["nc"] = nc
    return nc


def host_inputs(x, G, Wq, Wk, Wv, Wo, bo, b_extra):
    """Build the per-core input maps (layout prep + query-row sharding)."""
    import ml_dtypes

    f = np.float32
    bf = ml_dtypes.bfloat16
    x = np.asarray(x, f)
    G = np.asarray(G, f)
    xt = np.ascontiguousarray(x.transpose(0, 2, 1)).reshape(B, 2, 128, S).astype(bf)
    wq = np.ascontiguousarray(np.asarray(Wq, f).T * SCALE).reshape(2, 128, 256).astype(bf)
    wk = np.ascontiguousarray(np.asarray(Wk, f).T).reshape(2, 128, 256).astype(bf)
    wvt = np.asarray(Wv, f).T  # [256 in, 256 out]
    wv = np.zeros((2, 128, 260), f)
    for hp in range(2):
        wv[:, :, hp * 130:hp * 130 + 64] = wvt[:, hp * 128:hp * 128 + 64].reshape(2, 128, 64)
        wv[:, :, hp * 130 + 65:hp * 130 + 129] = wvt[:, hp * 128 + 64:hp * 128 + 128].reshape(2, 128, 64)
    wv = wv.astype(bf)
    wo = np.ascontiguousarray(np.asarray(Wo, f).T).reshape(H, 64, 256).astype(bf)

    shared = {"xt": xt, "wq": wq, "wk": wk, "wv": wv, "wo": wo}
    in_maps = []
    for c in range(NCORES):
        q0 = c * QR
        m = dict(shared)
        # gt[p, kc, q] = G[q0+q, kc*128+p]  (f32)
        gslice = np.ascontiguousarray(G[q0:q0 + QR, :].T)            # [S, QR]
        m["gt"] = np.ascontiguousarray(gslice.reshape(KC, 128, QR).transpose(1, 0, 2))
        m["xqt"] = np.ascontiguousarray(xt[:, :, :, q0:q0 + QR])
        in_maps.append(m)
    return in_maps


def run(in_maps, trace=False):
    from concourse.bass_utils import run_bass_kernel_spmd

    nc = build_bass()
    return run_bass_kernel_spmd(nc, in_maps, core_ids=list(range(NCORES)), trace=trace)


def kernel(x, G, Wq, Wk, Wv, Wo, bo, b_extra):
    in_maps = host_inputs(x, G, Wq, Wk, Wv, Wo, bo, b_extra)
    res = run(in_maps, trace=False)
    out = np.concatenate([res.results[c]["out"] for c in range(NCORES)], axis=1)
    bias = (np.asarray(bo, np.float32) + np.asarray(b_extra, np.float32)).reshape(1, 1, 256)
    return out.astype(np.float32) + bias
